# revision 1
# baseline (speedup 1.0000x reference)
"""Trainium2 Bass kernel for nn_CorrBlock_cascade (self-contained).

Pipeline (per core, core i handles clip/segment i = frames 8i..8i+7):
  conv21 (1x1, 64->16) -> BN21(relu) -> temporal shift -> 7x7 local corr
  -> BN22(relu) -> conv22 (1x1, 49->64) -> BN23 -> +residual -> relu
BN statistics are all-reduced across the 8 cores.

Device layouts:
  x / z / out : [128 = (f2, c), 3136] per frame-pair (4 pairs)
  y / a / products / corr rounds : [128 = (f, cm), 3136]
  corr2 (conv22 rhs) : [98 = (f2, k), 4*3136]
"""

import os
import numpy as np
import ml_dtypes

import concourse.bacc as bacc
import concourse.bass as bass
import concourse.mybir as mybir
from concourse import tile
from concourse.bass_utils import run_bass_kernel_spmd

N_CORES = 8
NT, C, H, W = 64, 64, 56, 56
CM = C // 4                  # 16
F = NT // N_CORES            # 8 frames per core
P = H * W                    # 3136
WPAD = 62                    # 56 + 2*3
PPAD = WPAD * WPAD           # 3844
BPAD_ALLOC = 3908            # padded alloc so shifted [56,62] views stay in-bounds
KK = 49
NTOT = float(NT * P)         # BN sample count per channel (global)
EPS = 1e-5
NCH = 7
CHUNK = P // NCH             # 448
ROUNDS = [16, 16, 16, 1]     # 49 offsets in 4 matmul-accumulation rounds
DT = mybir.dt
BF16 = ml_dtypes.bfloat16


def _build_nc(dbg=False):
    nc = bacc.Bacc("TRN2", target_bir_lowering=False, debug=False,
                   num_devices=N_CORES)
    dbg_tensors = {}

    def dump(name, sb_tile, shape, dtype):
        if not dbg:
            return
        d = nc.dram_tensor(f"dbg_{name}", shape, dtype, kind="ExternalOutput")
        dbg_tensors[name] = d
        nc.sync.dma_start(d[:], sb_tile)

    x4_d = nc.dram_tensor("x4", [4, 128, P], DT.float32, kind="ExternalInput")
    w21bd_d = nc.dram_tensor("w21bd", [128, 32], DT.bfloat16, kind="ExternalInput")
    w22bd_d = nc.dram_tensor("w22bd", [98, 128], DT.bfloat16, kind="ExternalInput")
    selred_d = nc.dram_tensor("selred", [128, 16 * 128], DT.bfloat16,
                              kind="ExternalInput")
    selb16_d = nc.dram_tensor("selb16", [128, 128], DT.float32, kind="ExternalInput")
    selb64_d = nc.dram_tensor("selb64", [128, 128], DT.float32, kind="ExternalInput")
    selbk_d = nc.dram_tensor("selbk", [128, 4 * 98], DT.float32, kind="ExternalInput")
    bnc128_d = nc.dram_tensor("bnc128", [128, 4], DT.float32, kind="ExternalInput")
    bnc98_d = nc.dram_tensor("bnc98", [98, 2], DT.float32, kind="ExternalInput")
    out_d = nc.dram_tensor("out", [4, 128, P], DT.float32, kind="ExternalOutput")

    RELU = mybir.ActivationFunctionType.Relu
    COPY = mybir.ActivationFunctionType.Copy
    SQRT = mybir.ActivationFunctionType.Sqrt
    MULT = mybir.AluOpType.mult
    ADD = mybir.AluOpType.add

    with tile.TileContext(nc) as tc:
        with (
            tc.tile_pool(name="const", bufs=1) as cpool,
            tc.tile_pool(name="big", bufs=1) as bpool,
            tc.tile_pool(name="work", bufs=1) as wpool,
            tc.tile_pool(name="prod", bufs=3) as ppool,
            tc.tile_pool(name="small", bufs=1) as spool,
            tc.tile_pool(name="psum", bufs=1, space="PSUM") as pspool,
            tc.tile_pool(name="dram", bufs=1, space="DRAM") as dpool,
        ):
            # ---- load constants ----
            w21bd = cpool.tile([128, 32], DT.bfloat16)
            w22bd = cpool.tile([98, 128], DT.bfloat16)
            selred = cpool.tile([128, 16 * 128], DT.bfloat16)
            selb16 = cpool.tile([128, 128], DT.float32)
            selb64 = cpool.tile([128, 128], DT.float32)
            selbk = cpool.tile([128, 4 * 98], DT.float32)
            bnc128 = cpool.tile([128, 4], DT.float32)
            bnc98 = cpool.tile([98, 2], DT.float32)
            for sb_t, dr_t in [(w21bd, w21bd_d), (w22bd, w22bd_d),
                               (selred, selred_d), (selb16, selb16_d),
                               (selb64, selb64_d), (selbk, selbk_d),
                               (bnc128, bnc128_d), (bnc98, bnc98_d)]:
                nc.sync.dma_start(sb_t[:], dr_t[:])

            # ---- load x (fp32 -> bf16 cast in DMA) ----
            x_all = bpool.tile([128, 4 * P], DT.bfloat16, tag="x")
            for p in range(4):
                nc.gpsimd.dma_start(x_all[:, p * P:(p + 1) * P], x4_d[p])

            # ---- conv21: y[(f,cm), pix] ----
            psum_y = pspool.tile([128, NCH, 512], DT.float32, tag="bigps")
            for p in range(4):
                for ch in range(NCH):
                    nc.tensor.matmul(
                        psum_y[32 * p:32 * p + 32, ch, 0:CHUNK],
                        w21bd[:], x_all[:, p * P + ch * CHUNK:p * P + (ch + 1) * CHUNK],
                        start=True, stop=True, tile_position=(0, 32 * p))

            # drain + BN21 stats
            y_sb = wpool.tile([128, P], DT.bfloat16, tag="y")
            st1 = spool.tile([128, 8], DT.float32)
            for ch in range(NCH):
                sl = slice(ch * CHUNK, (ch + 1) * CHUNK)
                nc.scalar.activation(y_sb[:, sl], psum_y[:, ch, 0:CHUNK], COPY,
                                     accum_out=st1[:, ch:ch + 1])
            trash = ppool.tile([128, P], DT.bfloat16, tag="prod")
            nc.vector.scalar_tensor_tensor(trash[:], y_sb[:], 1.0, y_sb[:],
                                           op0=MULT, op1=MULT,
                                           accum_out=st1[:, 7:8])
            dump("y", y_sb[:], [128, P], DT.bfloat16)
            ar1 = spool.tile([128, 2], DT.float32)
            nc.vector.tensor_reduce(ar1[:, 0:1], st1[:, 0:7],
                                    axis=mybir.AxisListType.X, op=ADD)
            nc.vector.tensor_copy(ar1[:, 1:2], st1[:, 7:8])

            # AllReduce #1
            cc1i = dpool.tile([128, 2], DT.float32)
            cc1o = dpool.tile([128, 2], DT.float32, addr_space="Shared")
            nc.sync.dma_start(cc1i[:], ar1[:])
            nc.gpsimd.collective_compute(
                "AllReduce", ADD, replica_groups=[list(range(N_CORES))],
                ins=[cc1i.opt()], outs=[cc1o.opt()])
            ar1r = spool.tile([128, 2], DT.float32)
            nc.sync.dma_start(ar1r[:], cc1o[:])

            # BN21 coefficient vectors (per-partition, (f,cm) layout)
            def bn_vectors(npart, psum_st, gvec, bvec, pool):
                """psum_st [npart,2] = (sum, sumsq); returns (svec, tvec)."""
                mean = pool.tile([npart, 1], DT.float32, name=f"mean{nc.next_id()}")
                e2 = pool.tile([npart, 1], DT.float32, name=f"e2{nc.next_id()}")
                var = pool.tile([npart, 1], DT.float32, name=f"var{nc.next_id()}")
                std = pool.tile([npart, 1], DT.float32, name=f"std{nc.next_id()}")
                rstd = pool.tile([npart, 1], DT.float32, name=f"rstd{nc.next_id()}")
                svec = pool.tile([npart, 1], DT.float32, name=f"svec{nc.next_id()}")
                tv = pool.tile([npart, 1], DT.float32, name=f"tv{nc.next_id()}")
                tvec = pool.tile([npart, 1], DT.float32, name=f"tvec{nc.next_id()}")
                eps_t = pool.tile([npart, 1], DT.float32, name=f"eps{nc.next_id()}")
                nc.vector.memset(eps_t[:], EPS)
                nc.scalar.mul(mean[:], psum_st[:, 0:1], 1.0 / NTOT)
                nc.scalar.mul(e2[:], psum_st[:, 1:2], 1.0 / NTOT)
                nc.vector.tensor_mul(var[:], mean[:], mean[:])
                nc.vector.tensor_sub(var[:], e2[:], var[:])
                nc.scalar.activation(std[:], var[:], SQRT, bias=eps_t[:])
                nc.vector.reciprocal(rstd[:], std[:])
                nc.vector.tensor_mul(svec[:], gvec, rstd[:])
                nc.vector.tensor_mul(tv[:], mean[:], svec[:])
                nc.vector.tensor_sub(tvec[:], bvec, tv[:])
                return svec, tvec

            pst1 = pspool.tile([128, 2], DT.float32, tag="stps")
            nc.tensor.matmul(pst1[:], selb16[:], ar1r[:], start=True, stop=True)
            s21, t21 = bn_vectors(128, pst1, bnc128[:, 0:1], bnc128[:, 1:2], spool)

            dump("ar1r", ar1r[:], [128, 2], DT.float32)
            dump("s21", s21[:], [128, 1], DT.float32)
            dump("t21", t21[:], [128, 1], DT.float32)
            # BN21 apply + relu (in place on y_sb -> "a")
            nc.scalar.activation(y_sb[:], y_sb[:], RELU, bias=t21[:], scale=s21[:])
            a_bf = y_sb

            # ---- build padded, temporally-shifted b (and odd-parity copy) ----
            bpad = wpool.tile([128, BPAD_ALLOC], DT.bfloat16, tag="bpad")
            bpad1 = wpool.tile([128, BPAD_ALLOC], DT.bfloat16, tag="bpad1")
            nc.vector.memset(bpad[:], 0.0)
            nc.vector.memset(bpad1[:], 0.0)

            def interior(t, shift):
                # AP over [(y+3)*62 + (x+3) - shift] for y,x in [0,56)
                base = 3 * WPAD + 3 - shift
                v = t[:, base:base + 56 * WPAD]
                v = v.rearrange("p (y x) -> p y x", y=56, x=WPAD)
                return v[:, :, 0:56]

            # b frame f = a frame f+1 (last frame pairs with itself)
            a3d = a_bf[:].rearrange("p (y x) -> p y x", y=56, x=56)
            nc.sync.dma_start(interior(bpad, 0)[0:112], a3d[16:128])
            nc.sync.dma_start(interior(bpad, 0)[112:128], a3d[112:128])
            nc.sync.dma_start(interior(bpad1, 1)[0:112], a3d[16:128])
            nc.sync.dma_start(interior(bpad1, 1)[112:128], a3d[112:128])

            dump("a", a_bf[:], [128, P], DT.bfloat16)
            dump("bpad", bpad[:], [128, BPAD_ALLOC], DT.bfloat16)
            dump("bpad1", bpad1[:], [128, BPAD_ALLOC], DT.bfloat16)
            # ---- correlation: 49 shifted multiplies + PE group-reduce ----
            corr_all = bpool.tile([128, 4 * P], DT.bfloat16, tag="corr")
            st2 = spool.tile([128, 8], DT.float32)
            a3dv = a_bf[:].rearrange("p (y x) -> p y x", y=56, x=56)
            k = 0
            for r, nslots in enumerate(ROUNDS):
                psum_corr = pspool.tile([128, NCH, 512], DT.float32, tag="bigps",
                                        name=f"psc{r}")
                for s in range(nslots):
                    dy, dx = k // 7, k % 7
                    delta = WPAD * dy + dx
                    src, off = (bpad, delta) if delta % 2 == 0 else (bpad1, delta - 1)
                    bview = src[:, off:off + 56 * WPAD]
                    bview = bview.rearrange("p (y x) -> p y x", y=56, x=WPAD)
                    bview = bview[:, :, 0:56]
                    prod = ppool.tile([128, P], DT.bfloat16, tag="prod",
                                      name=f"prod{k}")
                    p3d = prod[:].rearrange("p (y x) -> p y x", y=56, x=56)
                    nc.vector.tensor_mul(p3d, a3dv, bview)
                    for ch in range(NCH):
                        sl = slice(ch * CHUNK, (ch + 1) * CHUNK)
                        nc.tensor.matmul(
                            psum_corr[:, ch, 0:CHUNK],
                            selred[:, 128 * s:128 * (s + 1)],
                            prod[:, sl],
                            start=(s == 0), stop=(s == nslots - 1))
                    k += 1
                # drain round r + BN22 stats
                csl = slice(r * P, (r + 1) * P)
                corr_v = corr_all[:, csl].rearrange("p (c x) -> p c x",
                                                    c=NCH, x=CHUNK)
                nc.scalar.activation(corr_v, psum_corr[:, :, 0:CHUNK], COPY,
                                     accum_out=st2[:, r:r + 1])
                trash2 = ppool.tile([128, P], DT.bfloat16, tag="prod",
                                    name=f"trash2_{r}")
                nc.vector.scalar_tensor_tensor(
                    trash2[:], corr_all[:, csl], 1.0, corr_all[:, csl],
                    op0=MULT, op1=MULT, accum_out=st2[:, 4 + r:5 + r])

            dump("corr", corr_all[:], [128, 4 * P], DT.bfloat16)
            dump("st2", st2[:], [128, 8], DT.float32)
            # AllReduce #2 (launch) + corr re-layout DMA (overlaps AR latency)
            cc2i = dpool.tile([128, 8], DT.float32)
            cc2o = dpool.tile([128, 8], DT.float32, addr_space="Shared")
            nc.sync.dma_start(cc2i[:], st2[:])
            nc.gpsimd.collective_compute(
                "AllReduce", ADD, replica_groups=[list(range(N_CORES))],
                ins=[cc2i.opt()], outs=[cc2o.opt()])
            ar2r = spool.tile([128, 8], DT.float32)
            nc.sync.dma_start(ar2r[:], cc2o[:])

            corr2 = bpool.tile([98, 4 * P], DT.bfloat16, tag="corr2")
            for r, nslots in enumerate(ROUNDS):
                for f in range(F):
                    src = corr_all[f:8 * (nslots - 1) + f + 1:8,
                                   r * P:(r + 1) * P]
                    dst = corr2[49 * (f % 2) + 16 * r:
                                49 * (f % 2) + 16 * r + nslots,
                                (f // 2) * P:(f // 2 + 1) * P]
                    nc.sync.dma_start(dst, src)

            dump("corr2pre", corr2[:], [98, 4 * P], DT.bfloat16)
            # BN22 vectors in (f2,k) layout
            pst2 = pspool.tile([98, 2], DT.float32, tag="stps", name="pst2")
            ar2v = ar2r[:].rearrange("p (s r) -> p r s", s=2, r=4)
            for r in range(4):
                nc.tensor.matmul(pst2[:], selbk[:, 98 * r:98 * (r + 1)],
                                 ar2v[:, r, :], start=(r == 0), stop=(r == 3))
            s22, t22 = bn_vectors(98, pst2, bnc98[:, 0:1], bnc98[:, 1:2], spool)

            # BN22 apply + relu, in place on corr2
            nc.scalar.activation(corr2[:], corr2[:], RELU,
                                 bias=t22[:], scale=s22[:])

            dump("s22", s22[:], [98, 1], DT.float32)
            dump("t22", t22[:], [98, 1], DT.float32)
            dump("corr2post", corr2[:], [98, 4 * P], DT.bfloat16)
            # ---- conv22 ----
            z_all = bpool.tile([128, 4 * P], DT.bfloat16, tag="z")
            st3 = spool.tile([128, 8], DT.float32)
            for p in range(4):
                psum_z = pspool.tile([128, NCH, 512], DT.float32, tag="bigps",
                                     name=f"psz{p}")
                for ch in range(NCH):
                    nc.tensor.matmul(
                        psum_z[:, ch, 0:CHUNK], w22bd[:],
                        corr2[:, p * P + ch * CHUNK:p * P + (ch + 1) * CHUNK],
                        start=True, stop=True)
                zsl = slice(p * P, (p + 1) * P)
                z_v = z_all[:, zsl].rearrange("p (c x) -> p c x", c=NCH, x=CHUNK)
                nc.scalar.activation(z_v, psum_z[:, :, 0:CHUNK], COPY,
                                     accum_out=st3[:, p:p + 1])
                trash3 = ppool.tile([128, P], DT.bfloat16, tag="prod",
                                    name=f"trash3_{p}")
                nc.vector.scalar_tensor_tensor(
                    trash3[:], z_all[:, zsl], 1.0, z_all[:, zsl],
                    op0=MULT, op1=MULT, accum_out=st3[:, 4 + p:5 + p])

            ar3 = spool.tile([128, 2], DT.float32)
            nc.vector.tensor_reduce(ar3[:, 0:1], st3[:, 0:4],
                                    axis=mybir.AxisListType.X, op=ADD)
            nc.vector.tensor_reduce(ar3[:, 1:2], st3[:, 4:8],
                                    axis=mybir.AxisListType.X, op=ADD)

            # AllReduce #3
            cc3i = dpool.tile([128, 2], DT.float32)
            cc3o = dpool.tile([128, 2], DT.float32, addr_space="Shared")
            nc.sync.dma_start(cc3i[:], ar3[:])
            nc.gpsimd.collective_compute(
                "AllReduce", ADD, replica_groups=[list(range(N_CORES))],
                ins=[cc3i.opt()], outs=[cc3o.opt()])
            ar3r = spool.tile([128, 2], DT.float32)
            nc.sync.dma_start(ar3r[:], cc3o[:])

            pst3 = pspool.tile([128, 2], DT.float32, tag="stps", name="pst3")
            nc.tensor.matmul(pst3[:], selb64[:], ar3r[:], start=True, stop=True)
            s23, t23 = bn_vectors(128, pst3, bnc128[:, 2:3], bnc128[:, 3:4], spool)

            dump("z", z_all[:], [128, 4 * P], DT.bfloat16)
            dump("s23", s23[:], [128, 1], DT.float32)
            dump("t23", t23[:], [128, 1], DT.float32)
            # ---- final: relu(s23*z + t23 + x) ----
            for p in range(4):
                zsl = slice(p * P, (p + 1) * P)
                tmp = ppool.tile([128, P], DT.bfloat16, tag="prod",
                                 name=f"fin{p}")
                nc.vector.scalar_tensor_tensor(
                    tmp[:], z_all[:, zsl], s23[:], x_all[:, zsl],
                    op0=MULT, op1=ADD)
                o32 = wpool.tile([128, P], DT.float32, tag="o32",
                                 name=f"o32_{p}")
                nc.scalar.activation(o32[:], tmp[:], RELU, bias=t23[:])
                nc.sync.dma_start(out_d[p], o32[:])

    nc.compile()
    nc._dbg_names = list(dbg_tensors)
    return nc


def _host_constants(w21, w22):
    w21bd = np.zeros((128, 32), BF16)
    for f2 in range(2):
        w21bd[64 * f2:64 * f2 + 64, 16 * f2:16 * f2 + 16] = w21.T.astype(BF16)
    w22bd = np.zeros((98, 128), BF16)
    for f2 in range(2):
        w22bd[49 * f2:49 * f2 + 49, 64 * f2:64 * f2 + 64] = w22.T.astype(BF16)

    selred = np.zeros((128, 16, 128), BF16)
    for s in range(16):
        for f in range(F):
            selred[16 * f:16 * f + 16, s, 8 * s + f] = 1.0 / CM
    selred = selred.reshape(128, 16 * 128)

    pidx = np.arange(128)
    selb16 = (pidx[:, None] % 16 == pidx[None, :] % 16).astype(np.float32)
    selb64 = (pidx[:, None] % 64 == pidx[None, :] % 64).astype(np.float32)

    selbk = np.zeros((4, 128, 98), np.float32)
    k = 0
    for r, nslots in enumerate(ROUNDS):
        for s in range(nslots):
            for f in range(F):
                for f2 in range(2):
                    selbk[r, 8 * s + f, 49 * f2 + 16 * r + s] = 1.0
            k += 1
    selbk = selbk.transpose(1, 0, 2).reshape(128, 4 * 98)
    return w21bd, w22bd, selred, selb16, selb64, selbk


_NC_CACHE = {}


def kernel(x, w21, w22, g21, b21, g22, b22, g23, b23, trace=False, dbg=False):
    x = np.asarray(x, np.float32)
    w21 = np.asarray(w21, np.float32)
    w22 = np.asarray(w22, np.float32)
    g21 = np.asarray(g21, np.float32); b21 = np.asarray(b21, np.float32)
    g22 = np.asarray(g22, np.float32); b22 = np.asarray(b22, np.float32)
    g23 = np.asarray(g23, np.float32); b23 = np.asarray(b23, np.float32)

    key = ("nc_dbg" if dbg else "nc")
    if key not in _NC_CACHE:
        _NC_CACHE[key] = _build_nc(dbg=dbg)
    nc = _NC_CACHE[key]

    w21bd, w22bd, selred, selb16, selb64, selbk = _host_constants(w21, w22)
    pidx = np.arange(128)
    bnc128 = np.stack([g21[pidx % 16], b21[pidx % 16],
                       g23[pidx % 64], b23[pidx % 64]], 1).astype(np.float32)
    kidx = np.arange(98) % 49
    bnc98 = np.stack([g22[kidx], b22[kidx]], 1).astype(np.float32)

    in_maps = []
    for i in range(N_CORES):
        x4 = np.ascontiguousarray(
            x[F * i:F * (i + 1)].reshape(4, 128, P), np.float32)
        in_maps.append({
            "x4": x4, "w21bd": w21bd, "w22bd": w22bd, "selred": selred,
            "selb16": selb16, "selb64": selb64, "selbk": selbk,
            "bnc128": bnc128, "bnc98": bnc98,
        })

    res = run_bass_kernel_spmd(nc, in_maps, core_ids=list(range(N_CORES)),
                               trace=trace)
    out = np.empty((NT, C, H, W), np.float32)
    for i in range(N_CORES):
        out[F * i:F * (i + 1)] = res.results[i]["out"].reshape(F, C, H, W)
    if dbg:
        return out, res
    if trace:
        return out, res
    return out



# revision 4
# speedup vs baseline: 1.0221x; 1.0221x over previous
"""Trainium2 Bass kernel for nn_CorrBlock_cascade (self-contained).

Pipeline (per core, core i handles clip/segment i = frames 8i..8i+7):
  conv21 (1x1, 64->16) -> BN21(relu) -> temporal shift -> 7x7 local corr
  -> BN22(relu) -> conv22 (1x1, 49->64) -> BN23 -> +residual -> relu
BN statistics are all-reduced across the 8 cores.

v2 layout/schedule notes:
  - dummy AllReduce at t=0 absorbs core launch skew + warms ncfw
  - temporal shift built by a PE permutation matmul on pre-BN y during AR1,
    BN applied in place on the padded copies afterwards (BN vectors are
    16-periodic in partition so the 16-partition shift leaves them invariant)
  - products packed 2-3 offsets per DVE op via hand-built strided APs
  - per-chunk PSUM tiles ([128,512] x8 banks) with incremental drains
  - per-round corr relayout DMA overlaps the next round
  - all sum/sumsq stats ride scalar-engine accumulators (Square act)
"""

import numpy as np
import ml_dtypes

import concourse.bacc as bacc
import concourse.bass as bass
import concourse.mybir as mybir
from concourse import tile
from concourse.bass_types import AP as APc
from concourse.bass_utils import run_bass_kernel_spmd

N_CORES = 8
NT, C, H, W = 64, 64, 56, 56
CM = C // 4                  # 16
F = NT // N_CORES            # 8 frames per core
P = H * W                    # 3136
WPAD = 62                    # 56 + 2*3
BPAD_ALLOC = 3908
KK = 49
NCH = 7
CHUNK = P // NCH             # 448
SCH = 8
SCHUNK = P // SCH            # 392 = 7 rows of 56 (row-aligned)
ROUNDS = [16, 16, 16, 1]
NTOT = float(NT * P)
EPS = 1e-5
DT = mybir.dt
BF16 = ml_dtypes.bfloat16

# per-dy product packs: (parity tile, [dx list]); emission order E1, O, E2
PACKS = [("E1", [0, 2]), ("O", [1, 3, 5]), ("E2", [4, 6])]
# dx -> (pack tag, index within pack)
DX2PACK = {0: ("E1", 0), 2: ("E1", 1), 1: ("O", 0), 3: ("O", 1), 5: ("O", 2),
           4: ("E2", 0), 6: ("E2", 1)}


def _build_nc(dbg=False):
    nc = bacc.Bacc("TRN2", target_bir_lowering=False, debug=False,
                   num_devices=N_CORES)

    x4_d = nc.dram_tensor("x4", [4, 128, P], DT.float32, kind="ExternalInput")
    w21bd_d = nc.dram_tensor("w21bd", [128, 32], DT.bfloat16, kind="ExternalInput")
    w22bd_d = nc.dram_tensor("w22bd", [98, 128], DT.bfloat16, kind="ExternalInput")
    shift_d = nc.dram_tensor("shift16", [128, 128], DT.bfloat16, kind="ExternalInput")
    selred_d = nc.dram_tensor("selred", [128, 16 * 128], DT.bfloat16,
                              kind="ExternalInput")
    selb16_d = nc.dram_tensor("selb16", [128, 128], DT.float32, kind="ExternalInput")
    selb64_d = nc.dram_tensor("selb64", [128, 128], DT.float32, kind="ExternalInput")
    selbk_d = nc.dram_tensor("selbk", [128, 4 * 98], DT.float32, kind="ExternalInput")
    bnc128_d = nc.dram_tensor("bnc128", [128, 4], DT.float32, kind="ExternalInput")
    bnc98_d = nc.dram_tensor("bnc98", [98, 2], DT.float32, kind="ExternalInput")
    out_d = nc.dram_tensor("out", [4, 128, P], DT.float32, kind="ExternalOutput")

    RELU = mybir.ActivationFunctionType.Relu
    COPY = mybir.ActivationFunctionType.Copy
    SQRT = mybir.ActivationFunctionType.Sqrt
    SQUARE = mybir.ActivationFunctionType.Square
    MULT = mybir.AluOpType.mult
    ADD = mybir.AluOpType.add
    RG = [list(range(N_CORES))]

    with tile.TileContext(nc) as tc:
        with (
            tc.tile_pool(name="const", bufs=1) as cpool,
            tc.tile_pool(name="big", bufs=1) as bpool,
            tc.tile_pool(name="work", bufs=1) as wpool,
            tc.tile_pool(name="out32", bufs=2) as opool,
            tc.tile_pool(name="small", bufs=1) as spool,
            tc.tile_pool(name="psum", bufs=8, space="PSUM") as pspool,
            tc.tile_pool(name="dram", bufs=1, space="DRAM") as dpool,
        ):
            def psum_tile(name):
                return pspool.tile([128, 512], DT.float32, tag="ch", name=name)

            # ---- dummy AllReduce: absorbs launch skew, warms ncfw ----
            cc0i = dpool.tile([128, 1], DT.float32, name="cc0i")
            cc0o = dpool.tile([128, 1], DT.float32, addr_space="Shared",
                              name="cc0o")
            nc.gpsimd.collective_compute(
                "AllReduce", ADD, replica_groups=RG,
                ins=[cc0i.opt()], outs=[cc0o.opt()])

            # ---- ACT table preload: pin the sqrt set (has copy/relu/square/sqrt)
            tbl_in = spool.tile([128, 1], DT.float32, name="tbl_in")
            tbl_out = spool.tile([128, 1], DT.float32, name="tbl_out")
            nc.vector.memset(tbl_in[:], 1.0)
            nc.scalar.activation(tbl_out[:], tbl_in[:], SQRT)

            # ---- constants ----
            w21bd = cpool.tile([128, 32], DT.bfloat16)
            w22bd = cpool.tile([98, 128], DT.bfloat16)
            shift16 = cpool.tile([128, 128], DT.bfloat16)
            selred = cpool.tile([128, 16 * 128], DT.bfloat16)
            selb16 = cpool.tile([128, 128], DT.float32)
            selb64 = cpool.tile([128, 128], DT.float32)
            selbk = cpool.tile([128, 4 * 98], DT.float32)
            bnc128 = cpool.tile([128, 4], DT.float32)
            bnc98 = cpool.tile([98, 2], DT.float32)
            for sb_t, dr_t in [(w21bd, w21bd_d), (shift16, shift_d),
                               (w22bd, w22bd_d), (selred, selred_d),
                               (selb16, selb16_d), (selb64, selb64_d),
                               (selbk, selbk_d), (bnc128, bnc128_d),
                               (bnc98, bnc98_d)]:
                nc.sync.dma_start(sb_t[:], dr_t[:])

            # ---- zero the padded buffers early ----
            bpad = wpool.tile([128, BPAD_ALLOC], DT.bfloat16, tag="bpad")
            bpad1 = wpool.tile([128, BPAD_ALLOC], DT.bfloat16, tag="bpad1")
            nc.vector.memset(bpad[:], 0.0)
            nc.vector.memset(bpad1[:], 0.0)

            # ---- load x (fp32 -> bf16 cast in DMA), 8 chunks, 2 queues ----
            x_all = bpool.tile([128, 4 * P], DT.bfloat16, tag="x")
            HP = P // 2
            di = 0
            for h in range(2):
                for p in range(4):
                    dst = x_all[:, p * P + h * HP:p * P + (h + 1) * HP]
                    src = x4_d[p][:, h * HP:(h + 1) * HP]
                    nc.gpsimd.dma_start(dst, src)
                    di += 1

            # ---- conv21: y[(f,cm), pix], 7 chunks x 4 pairs via PE tiling ----
            y_sb = wpool.tile([128, P], DT.bfloat16, tag="y")
            trashq = wpool.tile([128, 512], DT.bfloat16, tag="trashq")
            stS1 = spool.tile([128, NCH], DT.float32, name="stS1")
            stQ1 = spool.tile([128, NCH], DT.float32, name="stQ1")
            for ch in range(NCH):
                ps = psum_tile(f"ps21_{ch}")
                for p in range(4):
                    nc.tensor.matmul(
                        ps[32 * p:32 * p + 32, 0:CHUNK],
                        w21bd[:],
                        x_all[:, p * P + ch * CHUNK:p * P + (ch + 1) * CHUNK],
                        start=True, stop=True, tile_position=(0, 32 * p))
                sl = slice(ch * CHUNK, (ch + 1) * CHUNK)
                nc.scalar.activation(y_sb[:, sl], ps[:, 0:CHUNK], COPY,
                                     accum_out=stS1[:, ch:ch + 1])
                nc.scalar.activation(trashq[:, 0:CHUNK], y_sb[:, sl], SQUARE,
                                     accum_out=stQ1[:, ch:ch + 1])

            # ---- temporal shift on pre-BN y (PE permutation), during AR1 ----
            def interior(t, shift, rows=slice(0, 56)):
                base = 3 * WPAD + 3 - shift
                v = t[:, base:base + 56 * WPAD]
                v = v.rearrange("p (y x) -> p y x", y=56, x=WPAD)
                return v[:, rows, 0:56]

            for ch in range(SCH):
                ps = psum_tile(f"pssh_{ch}")
                nc.tensor.matmul(
                    ps[:, 0:SCHUNK], shift16[:],
                    y_sb[:, ch * SCHUNK:(ch + 1) * SCHUNK],
                    start=True, stop=True)
                rows = slice(7 * ch, 7 * ch + 7)
                nc.scalar.activation(interior(bpad, 0, rows),
                                     ps[:, 0:SCHUNK], COPY)
                nc.scalar.activation(interior(bpad1, 1, rows),
                                     ps[:, 0:SCHUNK], COPY)

            # ---- BN21 stats reduce + AllReduce #1 ----
            ar1 = spool.tile([128, 2], DT.float32, name="ar1")
            nc.vector.tensor_reduce(ar1[:, 0:1], stS1[:],
                                    axis=mybir.AxisListType.X, op=ADD)
            nc.vector.tensor_reduce(ar1[:, 1:2], stQ1[:],
                                    axis=mybir.AxisListType.X, op=ADD)
            cc1i = dpool.tile([128, 2], DT.float32, name="cc1i")
            cc1o = dpool.tile([128, 2], DT.float32, addr_space="Shared",
                              name="cc1o")
            nc.sync.dma_start(cc1i[:], ar1[:])
            nc.gpsimd.collective_compute(
                "AllReduce", ADD, replica_groups=RG,
                ins=[cc1i.opt()], outs=[cc1o.opt()])
            ar1r = spool.tile([128, 2], DT.float32, name="ar1r")
            nc.sync.dma_start(ar1r[:], cc1o[:])

            def bn_vectors(npart, psum_st, gvec, bvec, pool):
                """psum_st [npart,2] = (sum, sumsq); returns (svec, tvec)."""
                mean = pool.tile([npart, 1], DT.float32, name=f"mean{nc.next_id()}")
                e2 = pool.tile([npart, 1], DT.float32, name=f"e2{nc.next_id()}")
                var = pool.tile([npart, 1], DT.float32, name=f"var{nc.next_id()}")
                std = pool.tile([npart, 1], DT.float32, name=f"std{nc.next_id()}")
                rstd = pool.tile([npart, 1], DT.float32, name=f"rstd{nc.next_id()}")
                svec = pool.tile([npart, 1], DT.float32, name=f"svec{nc.next_id()}")
                tv = pool.tile([npart, 1], DT.float32, name=f"tv{nc.next_id()}")
                tvec = pool.tile([npart, 1], DT.float32, name=f"tvec{nc.next_id()}")
                eps_t = pool.tile([npart, 1], DT.float32, name=f"eps{nc.next_id()}")
                nc.vector.memset(eps_t[:], EPS)
                nc.scalar.mul(mean[:], psum_st[:, 0:1], 1.0 / NTOT)
                nc.scalar.mul(e2[:], psum_st[:, 1:2], 1.0 / NTOT)
                nc.vector.tensor_mul(var[:], mean[:], mean[:])
                nc.vector.tensor_sub(var[:], e2[:], var[:])
                nc.scalar.activation(std[:], var[:], SQRT, bias=eps_t[:])
                nc.vector.reciprocal(rstd[:], std[:])
                nc.vector.tensor_mul(svec[:], gvec, rstd[:])
                nc.vector.tensor_mul(tv[:], mean[:], svec[:])
                nc.vector.tensor_sub(tvec[:], bvec, tv[:])
                return svec, tvec

            pst1t = psum_tile("pst1")
            pst1 = pst1t[:, 0:2]
            nc.tensor.matmul(pst1, selb16[:], ar1r[:], start=True, stop=True)
            s21, t21 = bn_vectors(128, pst1, bnc128[:, 0:1], bnc128[:, 1:2], spool)

            # ---- BN21 apply + relu: y in place, then padded copies ----
            # (s21/t21 are 16-periodic in partition; shift by 16 is invariant)
            nc.scalar.activation(y_sb[:], y_sb[:], RELU, bias=t21[:], scale=s21[:])
            nc.scalar.activation(interior(bpad, 0), interior(bpad, 0), RELU,
                                 bias=t21[:], scale=s21[:])
            nc.scalar.activation(interior(bpad1, 1), interior(bpad1, 1), RELU,
                                 bias=t21[:], scale=s21[:])
            a_bf = y_sb

            # ---- correlation ----
            prod = {
                "E1": wpool.tile([128, 2 * P], DT.bfloat16, tag="prodE1",
                                 name="prodE1"),
                "E2": wpool.tile([128, 2 * P], DT.bfloat16, tag="prodE2",
                                 name="prodE2"),
                "O": wpool.tile([128, 3 * P], DT.bfloat16, tag="prodO",
                                name="prodO"),
            }
            corr_round = wpool.tile([128, P], DT.bfloat16, tag="corr")
            corr2 = bpool.tile([98, 4 * P], DT.bfloat16, tag="corr2")
            stS2 = [spool.tile([128, NCH], DT.float32, name=f"stS2_{r}")
                    for r in range(4)]
            stQ2 = [spool.tile([128, NCH], DT.float32, name=f"stQ2_{r}")
                    for r in range(4)]

            RSTART = [0, 16, 32, 48]

            def rnd_of(k):
                for r in range(3, -1, -1):
                    if k >= RSTART[r]:
                        return r, k - RSTART[r]

            def emit_packs(dy):
                for tag, dxs in PACKS:
                    n = len(dxs)
                    pt = prod[tag]
                    src = bpad if dxs[0] % 2 == 0 else bpad1
                    base = WPAD * dy + (dxs[0] - (dxs[0] % 2))
                    b_ap = APc(src[:].tensor, base,
                               [[BPAD_ALLOC, 128], [2, n], [WPAD, 56], [1, 56]])
                    a_ap = APc(a_bf[:].tensor, 0,
                               [[P, 128], [0, n], [56, 56], [1, 56]])
                    o_ap = pt[:, 0:n * P].rearrange(
                        "p (j y x) -> p j y x", j=n, y=56, x=56)
                    nc.vector.tensor_mul(o_ap, a_ap, b_ap)

            def prod_slice(k, ch):
                dx = k % 7
                tag, j = DX2PACK[dx]
                return prod[tag][:, j * P + ch * CHUNK:j * P + (ch + 1) * CHUNK]

            psum_rc = {}
            for k in range(KK):
                dy, dx = k // 7, k % 7
                if dx == 0:
                    emit_packs(dy)
                r, s = rnd_of(k)
                last = (s == ROUNDS[r] - 1)
                for ch in range(NCH):
                    if s == 0:
                        psum_rc[(r, ch)] = psum_tile(f"psc_{r}_{ch}")
                    nc.tensor.matmul(
                        psum_rc[(r, ch)][:, 0:CHUNK],
                        selred[:, 128 * s:128 * (s + 1)],
                        prod_slice(k, ch),
                        start=(s == 0), stop=last)
                    if last:
                        sl = slice(ch * CHUNK, (ch + 1) * CHUNK)
                        nc.scalar.activation(
                            corr_round[:, sl], psum_rc[(r, ch)][:, 0:CHUNK],
                            COPY, accum_out=stS2[r][:, ch:ch + 1])
                        nc.scalar.activation(
                            trashq[:, 0:CHUNK], corr_round[:, sl], SQUARE,
                            accum_out=stQ2[r][:, ch:ch + 1])
                if last:
                    # relayout round r into conv22 operand layout (overlaps
                    # the next round's compute)
                    nslots = ROUNDS[r]
                    for f in range(F):
                        src = corr_round[f:8 * (nslots - 1) + f + 1:8, :]
                        dst = corr2[49 * (f % 2) + 16 * r:
                                    49 * (f % 2) + 16 * r + nslots,
                                    (f // 2) * P:(f // 2 + 1) * P]
                        nc.sync.dma_start(dst, src)

            # ---- BN22 stats + AllReduce #2 ----
            st2 = spool.tile([128, 8], DT.float32, name="st2")
            for r in range(4):
                nc.vector.tensor_reduce(st2[:, r:r + 1], stS2[r][:],
                                        axis=mybir.AxisListType.X, op=ADD)
                nc.vector.tensor_reduce(st2[:, 4 + r:5 + r], stQ2[r][:],
                                        axis=mybir.AxisListType.X, op=ADD)
            cc2i = dpool.tile([128, 8], DT.float32, name="cc2i")
            cc2o = dpool.tile([128, 8], DT.float32, addr_space="Shared",
                              name="cc2o")
            nc.sync.dma_start(cc2i[:], st2[:])
            nc.gpsimd.collective_compute(
                "AllReduce", ADD, replica_groups=RG,
                ins=[cc2i.opt()], outs=[cc2o.opt()])
            ar2r = spool.tile([128, 8], DT.float32, name="ar2r")
            nc.sync.dma_start(ar2r[:], cc2o[:])

            pst2t = psum_tile("pst2")
            pst2 = pst2t[0:98, 0:2]
            ar2v = ar2r[:].rearrange("p (s r) -> p r s", s=2, r=4)
            for r in range(4):
                nc.tensor.matmul(pst2, selbk[:, 98 * r:98 * (r + 1)],
                                 ar2v[:, r, :], start=(r == 0), stop=(r == 3))
            s22, t22 = bn_vectors(98, pst2, bnc98[:, 0:1], bnc98[:, 1:2], spool)

            # ---- conv22 (BN22 apply pipelined per pair) ----
            z_all = bpool.tile([128, 4 * P], DT.bfloat16, tag="z")
            stS3 = [spool.tile([128, NCH], DT.float32, name=f"stS3_{p}")
                    for p in range(4)]
            stQ3 = [spool.tile([128, NCH], DT.float32, name=f"stQ3_{p}")
                    for p in range(4)]
            for p in range(4):
                csl = slice(p * P, (p + 1) * P)
                nc.scalar.activation(corr2[:, csl], corr2[:, csl], RELU,
                                     bias=t22[:], scale=s22[:])
                for ch in range(NCH):
                    ps = psum_tile(f"psz_{p}_{ch}")
                    nc.tensor.matmul(
                        ps[:, 0:CHUNK], w22bd[:],
                        corr2[:, p * P + ch * CHUNK:p * P + (ch + 1) * CHUNK],
                        start=True, stop=True)
                    zsl = slice(p * P + ch * CHUNK, p * P + (ch + 1) * CHUNK)
                    nc.scalar.activation(z_all[:, zsl], ps[:, 0:CHUNK], COPY,
                                         accum_out=stS3[p][:, ch:ch + 1])
                    nc.vector.scalar_tensor_tensor(
                        trashq[:, 0:CHUNK], z_all[:, zsl], 1.0, z_all[:, zsl],
                        op0=MULT, op1=MULT, accum_out=stQ3[p][:, ch:ch + 1])

            # ---- BN23 stats + AllReduce #3 ----
            st3 = spool.tile([128, 8], DT.float32, name="st3")
            for p in range(4):
                nc.vector.tensor_reduce(st3[:, p:p + 1], stS3[p][:],
                                        axis=mybir.AxisListType.X, op=ADD)
                nc.vector.tensor_reduce(st3[:, 4 + p:5 + p], stQ3[p][:],
                                        axis=mybir.AxisListType.X, op=ADD)
            ar3 = spool.tile([128, 2], DT.float32, name="ar3")
            nc.vector.tensor_reduce(ar3[:, 0:1], st3[:, 0:4],
                                    axis=mybir.AxisListType.X, op=ADD)
            nc.vector.tensor_reduce(ar3[:, 1:2], st3[:, 4:8],
                                    axis=mybir.AxisListType.X, op=ADD)
            cc3i = dpool.tile([128, 2], DT.float32, name="cc3i")
            cc3o = dpool.tile([128, 2], DT.float32, addr_space="Shared",
                              name="cc3o")
            nc.sync.dma_start(cc3i[:], ar3[:])
            nc.gpsimd.collective_compute(
                "AllReduce", ADD, replica_groups=RG,
                ins=[cc3i.opt()], outs=[cc3o.opt()])
            ar3r = spool.tile([128, 2], DT.float32, name="ar3r")
            nc.sync.dma_start(ar3r[:], cc3o[:])

            pst3t = psum_tile("pst3")
            pst3 = pst3t[:, 0:2]
            nc.tensor.matmul(pst3, selb64[:], ar3r[:], start=True, stop=True)
            s23, t23 = bn_vectors(128, pst3, bnc128[:, 2:3], bnc128[:, 3:4], spool)

            # ---- final: relu(s23*z + t23 + x) ----
            for p in range(4):
                zsl = slice(p * P, (p + 1) * P)
                z1 = wpool.tile([128, P], DT.bfloat16, tag="z1",
                                name=f"z1_{p}")
                nc.vector.tensor_scalar_mul(z1[:], z_all[:, zsl], s23[:])
                w_t = wpool.tile([128, P], DT.bfloat16, tag="wfin",
                                 name=f"wfin_{p}")
                nc.vector.tensor_add(w_t[:], z1[:], x_all[:, zsl])
                o32 = opool.tile([128, P], DT.float32, tag="o32",
                                 name=f"o32_{p}")
                nc.scalar.activation(o32[:], w_t[:], RELU, bias=t23[:])
                nc.sync.dma_start(out_d[p], o32[:])

    nc.compile()
    return nc


def _host_constants(w21, w22):
    w21bd = np.zeros((128, 32), BF16)
    for f2 in range(2):
        w21bd[64 * f2:64 * f2 + 64, 16 * f2:16 * f2 + 16] = w21.T.astype(BF16)
    w22bd = np.zeros((98, 128), BF16)
    for f2 in range(2):
        w22bd[49 * f2:49 * f2 + 49, 64 * f2:64 * f2 + 64] = w22.T.astype(BF16)

    shift16 = np.zeros((128, 128), BF16)
    for m in range(128):
        k = m + 16 if m < 112 else m
        shift16[k, m] = 1.0

    selred = np.zeros((128, 16, 128), BF16)
    for s in range(16):
        for f in range(F):
            selred[16 * f:16 * f + 16, s, 8 * s + f] = 1.0 / CM
    selred = selred.reshape(128, 16 * 128)

    pidx = np.arange(128)
    selb16 = (pidx[:, None] % 16 == pidx[None, :] % 16).astype(np.float32)
    selb64 = (pidx[:, None] % 64 == pidx[None, :] % 64).astype(np.float32)

    selbk = np.zeros((4, 128, 98), np.float32)
    for r, nslots in enumerate(ROUNDS):
        for s in range(nslots):
            for f in range(F):
                for f2 in range(2):
                    selbk[r, 8 * s + f, 49 * f2 + 16 * r + s] = 1.0
    selbk = selbk.transpose(1, 0, 2).reshape(128, 4 * 98)
    return w21bd, w22bd, shift16, selred, selb16, selb64, selbk


_NC_CACHE = {}


def kernel(x, w21, w22, g21, b21, g22, b22, g23, b23, trace=False, dbg=False):
    x = np.asarray(x, np.float32)
    w21 = np.asarray(w21, np.float32)
    w22 = np.asarray(w22, np.float32)
    g21 = np.asarray(g21, np.float32); b21 = np.asarray(b21, np.float32)
    g22 = np.asarray(g22, np.float32); b22 = np.asarray(b22, np.float32)
    g23 = np.asarray(g23, np.float32); b23 = np.asarray(b23, np.float32)

    if "nc" not in _NC_CACHE:
        _NC_CACHE["nc"] = _build_nc()
    nc = _NC_CACHE["nc"]

    w21bd, w22bd, shift16, selred, selb16, selb64, selbk = _host_constants(
        w21, w22)
    pidx = np.arange(128)
    bnc128 = np.stack([g21[pidx % 16], b21[pidx % 16],
                       g23[pidx % 64], b23[pidx % 64]], 1).astype(np.float32)
    kidx = np.arange(98) % 49
    bnc98 = np.stack([g22[kidx], b22[kidx]], 1).astype(np.float32)

    in_maps = []
    for i in range(N_CORES):
        x4 = np.ascontiguousarray(
            x[F * i:F * (i + 1)].reshape(4, 128, P), np.float32)
        in_maps.append({
            "x4": x4, "w21bd": w21bd, "w22bd": w22bd, "shift16": shift16,
            "selred": selred, "selb16": selb16, "selb64": selb64,
            "selbk": selbk, "bnc128": bnc128, "bnc98": bnc98,
        })

    res = run_bass_kernel_spmd(nc, in_maps, core_ids=list(range(N_CORES)),
                               trace=trace)
    out = np.empty((NT, C, H, W), np.float32)
    for i in range(N_CORES):
        out[F * i:F * (i + 1)] = res.results[i]["out"].reshape(F, C, H, W)
    if trace:
        return out, res
    return out


# revision 11
# speedup vs baseline: 1.1668x; 1.1416x over previous
"""Trainium2 Bass kernel for nn_CorrBlock_cascade (self-contained).

Pipeline (per core, core i handles clip/segment i = frames 8i..8i+7):
  conv21 (1x1, 64->16) -> BN21(relu) -> temporal shift -> 7x7 local corr
  -> BN22(relu) -> conv22 (1x1, 49->64) -> BN23 -> +residual -> relu
BN statistics are all-reduced across the 8 cores.

v2 layout/schedule notes:
  - dummy AllReduce at t=0 absorbs core launch skew + warms ncfw
  - temporal shift built by a PE permutation matmul on pre-BN y during AR1,
    BN applied in place on the padded copies afterwards (BN vectors are
    16-periodic in partition so the 16-partition shift leaves them invariant)
  - products packed 2-3 offsets per DVE op via hand-built strided APs
  - per-chunk PSUM tiles ([128,512] x8 banks) with incremental drains
  - per-round corr relayout DMA overlaps the next round
  - all sum/sumsq stats ride scalar-engine accumulators (Square act)
"""

import numpy as np
import ml_dtypes

import concourse.bacc as bacc
import concourse.bass as bass
import concourse.mybir as mybir
from concourse import tile
from concourse.bass_types import AP as APc
from concourse.bass_utils import run_bass_kernel_spmd

N_CORES = 8
NT, C, H, W = 64, 64, 56, 56
CM = C // 4                  # 16
F = NT // N_CORES            # 8 frames per core
P = H * W                    # 3136
WPAD = 62                    # 56 + 2*3
BPAD_ALLOC = 3908
KK = 49
NCH = 7
CHUNK = P // NCH             # 448
SCH = 8
SCHUNK = P // SCH            # 392 = 7 rows of 56 (row-aligned)
ROUNDS = [16, 16, 16, 1]
NTOT = float(NT * P)
EPS = 1e-5
DT = mybir.dt
BF16 = ml_dtypes.bfloat16

# per-dy product packs: (parity tile, [dx list]); emission order E1, O, E2
PACKS = [("E1", [0, 2]), ("O", [1, 3, 5]), ("E2", [4, 6])]
# dx -> (pack tag, index within pack)
DX2PACK = {0: ("E1", 0), 2: ("E1", 1), 1: ("O", 0), 3: ("O", 1), 5: ("O", 2),
           4: ("E2", 0), 6: ("E2", 1)}


def _build_nc(dbg=False):
    nc = bacc.Bacc("TRN2", target_bir_lowering=False, debug=False,
                   num_devices=N_CORES)

    x4_d = nc.dram_tensor("x4", [4, 128, P], DT.float32, kind="ExternalInput")
    w21bd_d = nc.dram_tensor("w21bd", [128, 32], DT.bfloat16, kind="ExternalInput")
    w22bd_d = nc.dram_tensor("w22bd", [98, 128], DT.bfloat16, kind="ExternalInput")
    shift_d = nc.dram_tensor("shift16", [128, 128], DT.bfloat16, kind="ExternalInput")
    selred_d = nc.dram_tensor("selred", [128, 16 * 128], DT.bfloat16,
                              kind="ExternalInput")
    selb16_d = nc.dram_tensor("selb16", [128, 128], DT.float32, kind="ExternalInput")
    selb64_d = nc.dram_tensor("selb64", [128, 128], DT.float32, kind="ExternalInput")
    selbk_d = nc.dram_tensor("selbk", [128, 4 * 98], DT.float32, kind="ExternalInput")
    bnc128_d = nc.dram_tensor("bnc128", [128, 4], DT.float32, kind="ExternalInput")
    bnc98_d = nc.dram_tensor("bnc98", [98, 2], DT.float32, kind="ExternalInput")
    out_d = nc.dram_tensor("out", [4, 128, P], DT.float32, kind="ExternalOutput")

    RELU = mybir.ActivationFunctionType.Relu
    COPY = mybir.ActivationFunctionType.Copy
    SQRT = mybir.ActivationFunctionType.Sqrt
    SQUARE = mybir.ActivationFunctionType.Square
    MULT = mybir.AluOpType.mult
    ADD = mybir.AluOpType.add
    RG = [list(range(N_CORES))]

    with tile.TileContext(nc) as tc:
        with (
            tc.tile_pool(name="const", bufs=1) as cpool,
            tc.tile_pool(name="big", bufs=1) as bpool,
            tc.tile_pool(name="work", bufs=1) as wpool,
            tc.tile_pool(name="out32", bufs=2) as opool,
            tc.tile_pool(name="small", bufs=1) as spool,
            tc.tile_pool(name="psum", bufs=8, space="PSUM") as pspool,
            tc.tile_pool(name="dram", bufs=1, space="DRAM") as dpool,
        ):
            def psum_tile(name):
                return pspool.tile([128, 512], DT.float32, tag="ch", name=name)

            # ---- ACT table preload: pin the sqrt set (has copy/relu/square/sqrt)
            tbl_in = spool.tile([128, 1], DT.float32, name="tbl_in")
            tbl_out = spool.tile([128, 1], DT.float32, name="tbl_out")
            nc.vector.memset(tbl_in[:], 1.0)
            nc.scalar.activation(tbl_out[:], tbl_in[:], SQRT)

            # ---- constants ----
            w21bd = cpool.tile([128, 32], DT.bfloat16)
            w22bd = cpool.tile([98, 128], DT.bfloat16)
            shift16 = cpool.tile([128, 128], DT.bfloat16)
            selred = cpool.tile([128, 16 * 128], DT.bfloat16)
            selb16 = cpool.tile([128, 128], DT.float32)
            selb64 = cpool.tile([128, 128], DT.float32)
            selbk = cpool.tile([128, 4 * 98], DT.float32)
            bnc128 = cpool.tile([128, 4], DT.float32)
            bnc98 = cpool.tile([98, 2], DT.float32)
            for sb_t, dr_t in [(w21bd, w21bd_d), (shift16, shift_d),
                               (w22bd, w22bd_d), (selred, selred_d),
                               (selb16, selb16_d), (selb64, selb64_d),
                               (selbk, selbk_d), (bnc128, bnc128_d),
                               (bnc98, bnc98_d)]:
                nc.sync.dma_start(sb_t[:], dr_t[:])

            # ---- zero the padded buffers early ----
            bpad = wpool.tile([128, BPAD_ALLOC], DT.bfloat16, tag="bpad")
            bpad1 = wpool.tile([128, BPAD_ALLOC], DT.bfloat16, tag="bpad1")
            nc.vector.memset(bpad[:], 0.0)
            nc.vector.memset(bpad1[:], 0.0)

            # ---- load x (fp32 -> bf16 cast in DMA), 8 chunks, 2 queues ----
            x_all = bpool.tile([128, 4 * P], DT.bfloat16, tag="x")
            HP = P // 2
            for h in range(2):
                for p in range(4):
                    dst = x_all[:, p * P + h * HP:p * P + (h + 1) * HP]
                    src = x4_d[p][:, h * HP:(h + 1) * HP]
                    nc.gpsimd.dma_start(dst, src)

            # ---- conv21: y[(f,cm), pix], 7 chunks x 4 pairs via PE tiling ----
            y_sb = wpool.tile([128, P], DT.bfloat16, tag="y")
            trashq = wpool.tile([128, 512], DT.bfloat16, tag="trashq")
            stS1 = spool.tile([128, NCH], DT.float32, name="stS1")
            stQ1 = spool.tile([128, NCH], DT.float32, name="stQ1")
            for ch in range(NCH):
                ps = psum_tile(f"ps21_{ch}")
                for p in range(4):
                    nc.tensor.matmul(
                        ps[32 * p:32 * p + 32, 0:CHUNK],
                        w21bd[:],
                        x_all[:, p * P + ch * CHUNK:p * P + (ch + 1) * CHUNK],
                        start=True, stop=True, tile_position=(0, 32 * p))
                sl = slice(ch * CHUNK, (ch + 1) * CHUNK)
                nc.scalar.activation(y_sb[:, sl], ps[:, 0:CHUNK], COPY,
                                     accum_out=stS1[:, ch:ch + 1])
                nc.scalar.activation(trashq[:, 0:CHUNK], y_sb[:, sl], SQUARE,
                                     accum_out=stQ1[:, ch:ch + 1])

            # ---- temporal shift on pre-BN y (PE permutation), during AR1 ----
            def interior(t, shift, rows=slice(0, 56)):
                base = 3 * WPAD + 3 - shift
                v = t[:, base:base + 56 * WPAD]
                v = v.rearrange("p (y x) -> p y x", y=56, x=WPAD)
                return v[:, rows, 0:56]

            for ch in range(SCH):
                ps = psum_tile(f"pssh_{ch}")
                nc.tensor.matmul(
                    ps[:, 0:SCHUNK], shift16[:],
                    y_sb[:, ch * SCHUNK:(ch + 1) * SCHUNK],
                    start=True, stop=True)
                rows = slice(7 * ch, 7 * ch + 7)
                nc.scalar.activation(interior(bpad, 0, rows),
                                     ps[:, 0:SCHUNK], COPY)
                nc.scalar.activation(interior(bpad1, 1, rows),
                                     ps[:, 0:SCHUNK], COPY)

            # ---- BN21 stats reduce + AllReduce #1 ----
            ar1 = spool.tile([128, 2], DT.float32, name="ar1")
            nc.vector.tensor_reduce(ar1[:, 0:1], stS1[:],
                                    axis=mybir.AxisListType.X, op=ADD)
            nc.vector.tensor_reduce(ar1[:, 1:2], stQ1[:],
                                    axis=mybir.AxisListType.X, op=ADD)
            cc1i = dpool.tile([128, 2], DT.float32, name="cc1i")
            cc1o = dpool.tile([128, 2], DT.float32, addr_space="Shared",
                              name="cc1o")
            nc.sync.dma_start(cc1i[:], ar1[:])
            nc.gpsimd.collective_compute(
                "AllReduce", ADD, replica_groups=RG,
                ins=[cc1i.opt()], outs=[cc1o.opt()])
            ar1r = spool.tile([128, 2], DT.float32, name="ar1r")
            nc.sync.dma_start(ar1r[:], cc1o[:])

            def bn_vectors(npart, psum_st, gvec, bvec, pool):
                """psum_st [npart,2] = (sum, sumsq); returns (svec, tvec)."""
                mean = pool.tile([npart, 1], DT.float32, name=f"mean{nc.next_id()}")
                e2 = pool.tile([npart, 1], DT.float32, name=f"e2{nc.next_id()}")
                var = pool.tile([npart, 1], DT.float32, name=f"var{nc.next_id()}")
                std = pool.tile([npart, 1], DT.float32, name=f"std{nc.next_id()}")
                rstd = pool.tile([npart, 1], DT.float32, name=f"rstd{nc.next_id()}")
                svec = pool.tile([npart, 1], DT.float32, name=f"svec{nc.next_id()}")
                tv = pool.tile([npart, 1], DT.float32, name=f"tv{nc.next_id()}")
                tvec = pool.tile([npart, 1], DT.float32, name=f"tvec{nc.next_id()}")
                eps_t = pool.tile([npart, 1], DT.float32, name=f"eps{nc.next_id()}")
                nc.vector.memset(eps_t[:], EPS)
                nc.scalar.mul(mean[:], psum_st[:, 0:1], 1.0 / NTOT)
                nc.scalar.mul(e2[:], psum_st[:, 1:2], 1.0 / NTOT)
                nc.vector.tensor_mul(var[:], mean[:], mean[:])
                nc.vector.tensor_sub(var[:], e2[:], var[:])
                nc.scalar.activation(std[:], var[:], SQRT, bias=eps_t[:])
                nc.vector.reciprocal(rstd[:], std[:])
                nc.vector.tensor_mul(svec[:], gvec, rstd[:])
                nc.vector.tensor_mul(tv[:], mean[:], svec[:])
                nc.vector.tensor_sub(tvec[:], bvec, tv[:])
                return svec, tvec

            pst1t = psum_tile("pst1")
            pst1 = pst1t[:, 0:2]
            nc.tensor.matmul(pst1, selb16[:], ar1r[:], start=True, stop=True)
            s21, t21 = bn_vectors(128, pst1, bnc128[:, 0:1], bnc128[:, 1:2], spool)

            # ---- BN21 apply + relu: y in place, then padded copies ----
            # (s21/t21 are 16-periodic in partition; shift by 16 is invariant)
            # y and bpad on ACT; bpad1 on DVE (affine via broadcast-t21, then
            # relu via max) so the three applies overlap across engines.
            nc.scalar.activation(y_sb[:], y_sb[:], RELU, bias=t21[:], scale=s21[:])
            nc.scalar.activation(interior(bpad, 0), interior(bpad, 0), RELU,
                                 bias=t21[:], scale=s21[:])
            t21bc = APc(t21[:].tensor, 0, [[1, 128], [0, 56], [0, 56]])
            bp1v = interior(bpad1, 1)
            nc.vector.scalar_tensor_tensor(bp1v, bp1v, s21[:], t21bc,
                                           op0=MULT, op1=ADD)
            nc.vector.tensor_scalar_max(bp1v, bp1v, 0.0)
            a_bf = y_sb

            # ---- correlation ----
            # pack buffers double-buffered by dy parity; the B set lives in
            # scratch unions that later become z (S1) and z1/wfin (S2)
            S1 = bpool.tile([128, 4 * P], DT.bfloat16, tag="S1", name="S1")
            S2 = bpool.tile([128, 3 * P], DT.bfloat16, tag="S2", name="S2")
            prodA = {
                "E1": wpool.tile([128, 2 * P], DT.bfloat16, tag="prodE1",
                                 name="prodE1"),
                "E2": wpool.tile([128, 2 * P], DT.bfloat16, tag="prodE2",
                                 name="prodE2"),
                "O": wpool.tile([128, 3 * P], DT.bfloat16, tag="prodO",
                                name="prodO"),
            }
            corr_round = wpool.tile([128, P], DT.bfloat16, tag="corr")
            corr2 = bpool.tile([98, 4 * P], DT.bfloat16, tag="corr2")
            stS2 = [spool.tile([128, NCH], DT.float32, name=f"stS2_{r}")
                    for r in range(4)]
            stQ2 = [spool.tile([128, NCH], DT.float32, name=f"stQ2_{r}")
                    for r in range(4)]

            RSTART = [0, 16, 32, 48]

            def rnd_of(k):
                for r in range(3, -1, -1):
                    if k >= RSTART[r]:
                        return r, k - RSTART[r]

            def prod_view(dy, tag):
                if dy % 2 == 0:
                    return prodA[tag][:]
                if tag == "E1":
                    return S1[:, 0:2 * P]
                if tag == "E2":
                    return S1[:, 2 * P:4 * P]
                return S2[:, 0:3 * P]

            def emit_packs(dy):
                for tag, dxs in PACKS:
                    n = len(dxs)
                    pt = prod_view(dy, tag)
                    src = bpad if dxs[0] % 2 == 0 else bpad1
                    base = WPAD * dy + (dxs[0] - (dxs[0] % 2))
                    b_ap = APc(src[:].tensor, base,
                               [[BPAD_ALLOC, 128], [2, n], [WPAD, 56], [1, 56]])
                    a_ap = APc(a_bf[:].tensor, 0,
                               [[P, 128], [0, n], [56, 56], [1, 56]])
                    o_ap = pt[:, 0:n * P].rearrange(
                        "p (j y x) -> p j y x", j=n, y=56, x=56)
                    nc.vector.tensor_mul(o_ap, a_ap, b_ap)

            def prod_slice(k, ch):
                dy, dx = k // 7, k % 7
                tag, j = DX2PACK[dx]
                pt = prod_view(dy, tag)
                return pt[:, j * P + ch * CHUNK:j * P + (ch + 1) * CHUNK]

            psum_rc = {}
            for k in range(KK):
                dy, dx = k // 7, k % 7
                if dx == 0:
                    emit_packs(dy)
                r, s = rnd_of(k)
                last = (s == ROUNDS[r] - 1)
                for ch in range(NCH):
                    if s == 0:
                        psum_rc[(r, ch)] = psum_tile(f"psc_{r}_{ch}")
                    nc.tensor.matmul(
                        psum_rc[(r, ch)][:, 0:CHUNK],
                        selred[:, 128 * s:128 * (s + 1)],
                        prod_slice(k, ch),
                        start=(s == 0), stop=last)
                    if last:
                        sl = slice(ch * CHUNK, (ch + 1) * CHUNK)
                        nc.scalar.activation(
                            corr_round[:, sl], psum_rc[(r, ch)][:, 0:CHUNK],
                            COPY, accum_out=stS2[r][:, ch:ch + 1])
                        nc.scalar.activation(
                            trashq[:, 0:CHUNK], corr_round[:, sl], SQUARE,
                            accum_out=stQ2[r][:, ch:ch + 1])
                if last:
                    # relayout round r into conv22 operand layout (overlaps
                    # the next round's compute)
                    nslots = ROUNDS[r]
                    for f in range(F):
                        src = corr_round[f:8 * (nslots - 1) + f + 1:8, :]
                        dst = corr2[49 * (f % 2) + 16 * r:
                                    49 * (f % 2) + 16 * r + nslots,
                                    (f // 2) * P:(f // 2 + 1) * P]
                        nc.sync.dma_start(dst, src)

            # ---- BN22 stats + AllReduce #2 ----
            st2 = spool.tile([128, 8], DT.float32, name="st2")
            for r in range(4):
                nc.vector.tensor_reduce(st2[:, r:r + 1], stS2[r][:],
                                        axis=mybir.AxisListType.X, op=ADD)
                nc.vector.tensor_reduce(st2[:, 4 + r:5 + r], stQ2[r][:],
                                        axis=mybir.AxisListType.X, op=ADD)
            cc2i = dpool.tile([128, 8], DT.float32, name="cc2i")
            cc2o = dpool.tile([128, 8], DT.float32, addr_space="Shared",
                              name="cc2o")
            nc.sync.dma_start(cc2i[:], st2[:])
            nc.gpsimd.collective_compute(
                "AllReduce", ADD, replica_groups=RG,
                ins=[cc2i.opt()], outs=[cc2o.opt()])
            ar2r = spool.tile([128, 8], DT.float32, name="ar2r")
            nc.sync.dma_start(ar2r[:], cc2o[:])

            pst2t = psum_tile("pst2")
            pst2 = pst2t[0:98, 0:2]
            ar2v = ar2r[:].rearrange("p (s r) -> p r s", s=2, r=4)
            for r in range(4):
                nc.tensor.matmul(pst2, selbk[:, 98 * r:98 * (r + 1)],
                                 ar2v[:, r, :], start=(r == 0), stop=(r == 3))
            s22, t22 = bn_vectors(98, pst2, bnc98[:, 0:1], bnc98[:, 1:2], spool)

            # ---- conv22 (BN22 apply pipelined per pair) ----
            # z lives in S1 (the dy-odd E pack buffers, dead by now);
            # psum drains on DVE, sumsq on ACT, applies on ACT.
            z_all = S1
            stS3 = [spool.tile([128, NCH], DT.float32, name=f"stS3_{p}")
                    for p in range(4)]
            stQ3 = [spool.tile([128, NCH], DT.float32, name=f"stQ3_{p}")
                    for p in range(4)]

            def bn22_apply(p):
                csl = slice(p * P, (p + 1) * P)
                nc.scalar.activation(corr2[:, csl], corr2[:, csl], RELU,
                                     bias=t22[:], scale=s22[:])

            bn22_apply(0)
            bn22_apply(1)
            for p in range(4):
                for ch in range(NCH):
                    ps = psum_tile(f"psz_{p}_{ch}")
                    nc.tensor.matmul(
                        ps[:, 0:CHUNK], w22bd[:],
                        corr2[:, p * P + ch * CHUNK:p * P + (ch + 1) * CHUNK],
                        start=True, stop=True)
                    zsl = slice(p * P + ch * CHUNK, p * P + (ch + 1) * CHUNK)
                    nc.vector.tensor_scalar(
                        z_all[:, zsl], ps[:, 0:CHUNK], 1.0, 0.0, op0=MULT,
                        op1=ADD, accum_out=stS3[p][:, ch:ch + 1])
                    nc.scalar.activation(trashq[:, 0:CHUNK], z_all[:, zsl],
                                         SQUARE,
                                         accum_out=stQ3[p][:, ch:ch + 1])
                if p + 2 < 4:
                    bn22_apply(p + 2)

            # ---- BN23 stats + AllReduce #3 ----
            st3 = spool.tile([128, 8], DT.float32, name="st3")
            for p in range(4):
                nc.vector.tensor_reduce(st3[:, p:p + 1], stS3[p][:],
                                        axis=mybir.AxisListType.X, op=ADD)
                nc.vector.tensor_reduce(st3[:, 4 + p:5 + p], stQ3[p][:],
                                        axis=mybir.AxisListType.X, op=ADD)
            ar3 = spool.tile([128, 2], DT.float32, name="ar3")
            nc.vector.tensor_reduce(ar3[:, 0:1], st3[:, 0:4],
                                    axis=mybir.AxisListType.X, op=ADD)
            nc.vector.tensor_reduce(ar3[:, 1:2], st3[:, 4:8],
                                    axis=mybir.AxisListType.X, op=ADD)
            cc3i = dpool.tile([128, 2], DT.float32, name="cc3i")
            cc3o = dpool.tile([128, 2], DT.float32, addr_space="Shared",
                              name="cc3o")
            nc.sync.dma_start(cc3i[:], ar3[:])
            nc.gpsimd.collective_compute(
                "AllReduce", ADD, replica_groups=RG,
                ins=[cc3i.opt()], outs=[cc3o.opt()])
            ar3r = spool.tile([128, 2], DT.float32, name="ar3r")
            nc.sync.dma_start(ar3r[:], cc3o[:])

            pst3t = psum_tile("pst3")
            pst3 = pst3t[:, 0:2]
            nc.tensor.matmul(pst3, selb64[:], ar3r[:], start=True, stop=True)
            s23, t23 = bn_vectors(128, pst3, bnc128[:, 2:3], bnc128[:, 3:4], spool)

            # ---- final: relu(s23*z + t23 + x) ----
            # z1/wfin live in S2 (the dy-odd O pack buffer, dead by now)
            for p in range(4):
                zsl = slice(p * P, (p + 1) * P)
                z1 = S2[:, 0:P] if p % 2 == 0 else S2[:, 2 * P:3 * P]
                nc.vector.tensor_scalar_mul(z1, z_all[:, zsl], s23[:])
                w_t = S2[:, P:2 * P]
                nc.vector.tensor_add(w_t, z1, x_all[:, zsl])
                o32 = opool.tile([128, P], DT.float32, tag="o32",
                                 name=f"o32_{p}")
                nc.scalar.activation(o32[:], w_t, RELU, bias=t23[:])
                if p % 2 == 0:
                    nc.sync.dma_start(out_d[p], o32[:])
                else:
                    nc.gpsimd.dma_start(out_d[p], o32[:])

    nc.compile()
    return nc


def _host_constants(w21, w22):
    w21bd = np.zeros((128, 32), BF16)
    for f2 in range(2):
        w21bd[64 * f2:64 * f2 + 64, 16 * f2:16 * f2 + 16] = w21.T.astype(BF16)
    w22bd = np.zeros((98, 128), BF16)
    for f2 in range(2):
        w22bd[49 * f2:49 * f2 + 49, 64 * f2:64 * f2 + 64] = w22.T.astype(BF16)

    shift16 = np.zeros((128, 128), BF16)
    for m in range(128):
        k = m + 16 if m < 112 else m
        shift16[k, m] = 1.0

    selred = np.zeros((128, 16, 128), BF16)
    for s in range(16):
        for f in range(F):
            selred[16 * f:16 * f + 16, s, 8 * s + f] = 1.0 / CM
    selred = selred.reshape(128, 16 * 128)

    pidx = np.arange(128)
    selb16 = (pidx[:, None] % 16 == pidx[None, :] % 16).astype(np.float32)
    selb64 = (pidx[:, None] % 64 == pidx[None, :] % 64).astype(np.float32)

    selbk = np.zeros((4, 128, 98), np.float32)
    for r, nslots in enumerate(ROUNDS):
        for s in range(nslots):
            for f in range(F):
                for f2 in range(2):
                    selbk[r, 8 * s + f, 49 * f2 + 16 * r + s] = 1.0
    selbk = selbk.transpose(1, 0, 2).reshape(128, 4 * 98)
    return w21bd, w22bd, shift16, selred, selb16, selb64, selbk


_NC_CACHE = {}


def kernel(x, w21, w22, g21, b21, g22, b22, g23, b23, trace=False, dbg=False):
    x = np.asarray(x, np.float32)
    w21 = np.asarray(w21, np.float32)
    w22 = np.asarray(w22, np.float32)
    g21 = np.asarray(g21, np.float32); b21 = np.asarray(b21, np.float32)
    g22 = np.asarray(g22, np.float32); b22 = np.asarray(b22, np.float32)
    g23 = np.asarray(g23, np.float32); b23 = np.asarray(b23, np.float32)

    if "nc" not in _NC_CACHE:
        _NC_CACHE["nc"] = _build_nc()
    nc = _NC_CACHE["nc"]

    w21bd, w22bd, shift16, selred, selb16, selb64, selbk = _host_constants(
        w21, w22)
    pidx = np.arange(128)
    bnc128 = np.stack([g21[pidx % 16], b21[pidx % 16],
                       g23[pidx % 64], b23[pidx % 64]], 1).astype(np.float32)
    kidx = np.arange(98) % 49
    bnc98 = np.stack([g22[kidx], b22[kidx]], 1).astype(np.float32)

    in_maps = []
    for i in range(N_CORES):
        x4 = np.ascontiguousarray(
            x[F * i:F * (i + 1)].reshape(4, 128, P), np.float32)
        in_maps.append({
            "x4": x4, "w21bd": w21bd, "w22bd": w22bd, "shift16": shift16,
            "selred": selred, "selb16": selb16, "selb64": selb64,
            "selbk": selbk, "bnc128": bnc128, "bnc98": bnc98,
        })

    res = run_bass_kernel_spmd(nc, in_maps, core_ids=list(range(N_CORES)),
                               trace=trace)
    out = np.empty((NT, C, H, W), np.float32)
    for i in range(N_CORES):
        out[F * i:F * (i + 1)] = res.results[i]["out"].reshape(F, C, H, W)
    if trace:
        return out, res
    return out
